# revision 1
# baseline (speedup 1.0000x reference)
"""Trainium2 Bass kernel for nn_AdaptiveGraphConvolutionalLSTM.

Reference computation (B=32, F=1024):
    gc_R  = concat_k( einsum('bf,bfg->bg', x, GC_Rk_w  * Rk_i) )   k=1..3
    gc_NR = concat_k( einsum('bf,bfg->bg', x, GC_NRk_w * Rk_i) )   (NR reuses R masks)
    combined = concat([gc_R, gc_NR, hidden])                        # [B, 7F]
    f,i,o = sigmoid(combined @ {fl,il,ol}_w.T + b); C = tanh(combined @ Cl_w.T + Cl_b)
    Cell = f*cell + i*C ; Hidden = o*tanh(Cell)

Distribution over 8 NeuronCores (beats plain batch-DP on memory traffic):
  - The GC output dim g and the gate output columns are sharded across
    cores (128 columns each); the NR*_i masks are never read (unused by
    the reference), so per-core HBM traffic is ~33 MiB instead of the
    ~184 MiB replicated-weight data parallelism would need.
  - Per (mask k, f-chunk): u = mask * w is computed elementwise in bf16
    (VectorE 2x mode, weight broadcast over batch); PE contracts over f
    with x^T as the stationary operand (M=16 half-batch), producing
    x*mask*w reduced over f for every (b, b', g); the useful diagonal
    b==b' is pulled out through a small DRAM bounce. GC psums are
    [16, 1024] half-g tiles so three can pipeline alongside the gate
    accumulator within the 8 PSUM banks.
  - After each (mask k, branch), an 8 KB/rank AllGather shares that
    chunk of combined^T, one xbar-transpose DMA flips it to [g, b]
    layout, and the corresponding gate matmuls run immediately
    (stationary = combined^T chunk, moving = column-sharded gate weight
    chunk), overlapping comm + gates under later compute. Gates, LSTM
    elementwise, and outputs stay in natural [b, col] layout; the gate
    bias is folded in as an extra contraction chunk against ones.

kernel(**inputs) takes the FULL inputs and returns (Hidden, Cell) full.
"""
import numpy as np
import ml_dtypes

from concourse import bass, bacc, tile, mybir
from concourse.bass_utils import run_bass_kernel_spmd

BF16 = ml_dtypes.bfloat16
B, F, K, NCORES = 32, 1024, 3, 8
P = 128          # partitions / f-chunk size
FC = F // P      # 8 f-chunks
G = F // NCORES  # 128 g-columns per core
GH = G // 2      # 64, half-g (GC psum free = HB*GH = 1024)
NKC = 7 * FC + 1  # 57 gate contraction chunks (48 gathered + 8 hidden + bias)
HB = B // 2      # 16, half batch (GC psum M)

_DT_BF = mybir.dt.bfloat16
_DT_F32 = mybir.dt.float32


def build_nc(reps: int = 1):
    """Build the SPMD per-core program. reps>1 repeats the whole compute
    body back-to-back inside one NEFF (for timing); reps=1 is the real
    kernel."""
    nc = bacc.Bacc("TRN2", target_bir_lowering=False, debug=False,
                   num_devices=NCORES)

    p_m = nc.dram_tensor("m", [K, FC, P, B * G], _DT_BF,
                         kind="ExternalInput")
    # w6 laid out k-major: block j = 2*k + br, so k's two weights are
    # adjacent and can be DMAed per-k
    p_w6 = nc.dram_tensor("w6", [K, P, 2 * FC * G], _DT_BF,
                          kind="ExternalInput")
    p_xT = nc.dram_tensor("xT", [P, FC * B], _DT_BF, kind="ExternalInput")
    p_hp = nc.dram_tensor("hp", [P, 9 * B], _DT_BF, kind="ExternalInput")
    p_gw = nc.dram_tensor("gw", [P, NKC * 4 * P], _DT_BF,
                          kind="ExternalInput")
    p_cs = nc.dram_tensor("cs", [B, G], _DT_F32, kind="ExternalInput")
    p_out = nc.dram_tensor("out", [2, B, G], _DT_F32, kind="ExternalOutput")

    AF = mybir.ActivationFunctionType
    with tile.TileContext(nc) as tc:
        with tc.tile_pool(name="mp", bufs=12) as mp, \
             tc.tile_pool(name="up", bufs=6) as up, \
             tc.tile_pool(name="cst", bufs=1) as cst, \
             tc.tile_pool(name="stg", bufs=3) as stg, \
             tc.tile_pool(name="ctp", bufs=3) as ctp, \
             tc.tile_pool(name="gwp", bufs=2) as gwp, \
             tc.tile_pool(name="sml", bufs=2) as sml, \
             tc.tile_pool(name="pgc", bufs=3, space="PSUM") as pgc, \
             tc.tile_pool(name="pgt", bufs=1, space="PSUM") as pgt, \
             tc.tile_pool(name="dsc", bufs=3, space="DRAM") as dsc, \
             tc.tile_pool(name="dcc", bufs=3, space="DRAM") as dcc, \
             tc.tile_pool(name="dgg", bufs=3, space="DRAM") as dgg:

            # loads needed first (k=0 compute): w6 k-chunks + x
            w6_t = cst.tile([P, K * 2 * FC * G], _DT_BF, tag="w6")
            nc.sync.dma_start(out=w6_t[:, 0:2 * FC * G], in_=p_w6[0, :, :])
            xT_t = cst.tile([P, FC * B], _DT_BF, tag="xT")
            nc.sync.dma_start(out=xT_t[:, :], in_=p_xT[:, :])

            def load_mask_tiles(k):
                tiles = []
                for j in range(FC):
                    mt = mp.tile([P, B * G], _DT_BF, tag="m", name="mt")
                    nc.sync.dma_start(out=mt[:, :], in_=p_m[k, j, :, :])
                    tiles.append(mt)
                return tiles

            m_tiles = load_mask_tiles(0)

            # remaining loads (emitted after k=0 masks so their DMAs queue
            # behind them)
            for k in range(1, K):
                nc.sync.dma_start(out=w6_t[:, k * 2 * FC * G:
                                           (k + 1) * 2 * FC * G],
                                  in_=p_w6[k, :, :])
            hp_t = cst.tile([P, 9 * B], _DT_BF, tag="hp")
            nc.sync.dma_start(out=hp_t[:, :], in_=p_hp[:, :])
            cs_t = cst.tile([B, G], _DT_F32, tag="cs")
            nc.sync.dma_start(out=cs_t[:, :], in_=p_cs[:, :])
            # gate weights packed (kc, gate, m): each contraction chunk is
            # one [128, 512] rhs covering all 4 gates. h+bias chunks
            # (48..56) stay resident; gathered-k chunks stream per mask k.
            gwh_t = cst.tile([P, 9 * 4 * P], _DT_BF, tag="gwh")
            nc.sync.dma_start(out=gwh_t[:, :], in_=p_gw[:, 48 * 4 * P:])

            def load_gw_k(k, pool):
                t = pool.tile([P, 16 * 4 * P], _DT_BF, tag="gwk", name="gwk")
                nc.sync.dma_start(
                    out=t[:, :],
                    in_=p_gw[:, k * 16 * 4 * P:(k + 1) * 16 * 4 * P])
                return t

            for rep in range(reps):
                if rep > 0:
                    m_tiles = load_mask_tiles(0)
                pg_t = pgt.tile([B, 4 * G], _DT_F32, tag="pg", name="pg")

                # ---- phase A: hidden-state + bias gate partials ----
                for kc in range(48, 57):
                    lhs = hp_t[:, (kc - 48) * B:(kc - 47) * B]
                    nc.tensor.matmul(
                        pg_t[:, :], lhsT=lhs,
                        rhs=gwh_t[:, (kc - 48) * 4 * P:(kc - 47) * 4 * P],
                        start=(kc == 48), stop=False)
                gw_k_tile = load_gw_k(0, gwp)

                # ---- main loop over (mask k, branch) ----
                for k in range(K):
                    for br in range(2):      # 0 = R branch, 1 = NR branch
                        w_j = 2 * k + br     # w6 block index (k-major)
                        contrib = dcc.tile([B, G], _DT_BF, tag="contrib",
                                           name="contrib")
                        for h in range(2):   # batch half
                            us = []
                            for fc in range(FC):
                                u = up.tile([P, HB * G], _DT_BF, tag="u",
                                            name="u")
                                w_ap = (w6_t[:, (w_j * FC + fc) * G:
                                             (w_j * FC + fc + 1) * G]
                                        .unsqueeze(1).broadcast_to([P, HB, G]))
                                moff = h * HB * G
                                m_ap = (m_tiles[fc][:, moff:moff + HB * G]
                                        .rearrange("p (b g) -> p b g", g=G))
                                u_ap = u[:, :].rearrange("p (b g) -> p b g",
                                                         g=G)
                                nc.vector.tensor_mul(u_ap, m_ap, w_ap)
                                us.append(u)
                            lhs_h = [xT_t[:, fc * B + h * HB:
                                          fc * B + (h + 1) * HB]
                                     for fc in range(FC)]
                            for gh in range(2):   # half-g psum tiles
                                psum = pgc.tile([HB, HB * GH], _DT_F32,
                                                tag="gc", name="gcps")
                                for fc in range(FC):
                                    u3 = us[fc][:, :].rearrange(
                                        "p (b g) -> p b g", g=G)
                                    for bh in range(2):
                                        rhs = u3[:, bh * 8:(bh + 1) * 8,
                                                 gh * GH:(gh + 1) * GH]
                                        nc.tensor.matmul(
                                            psum[:, bh * 8 * GH:
                                                 (bh + 1) * 8 * GH],
                                            lhsT=lhs_h[fc], rhs=rhs,
                                            start=(fc == 0),
                                            stop=(fc == FC - 1))
                                # extract diagonal b==b' via DRAM bounce
                                stage = stg.tile([HB, HB * GH], _DT_BF,
                                                 tag="stage", name="stage")
                                nc.scalar.activation(stage[:, :], psum[:, :],
                                                     AF.Copy)
                                scr = dsc.tile([HB, HB * GH], _DT_BF,
                                               tag="scr", name="scr")
                                nc.sync.dma_start(out=scr[:, :],
                                                  in_=stage[:, :])
                                scr_ap = scr[:, :]
                                diag = bass.AP(scr_ap.tensor, scr_ap.offset,
                                               [[HB * GH + GH, HB], [1, GH]])
                                nc.sync.dma_start(
                                    out=contrib[h * HB:(h + 1) * HB,
                                                gh * GH:(gh + 1) * GH],
                                    in_=diag)
                        # ---- AllGather this (k, br) chunk + its gates ----
                        gathered = dgg.tile([NCORES * B, G], _DT_BF,
                                            tag="gath", name="gath",
                                            addr_space="Shared")
                        nc.gpsimd.collective_compute(
                            "AllGather", mybir.AluOpType.bypass,
                            replica_groups=[list(range(NCORES))],
                            ins=[contrib.opt()], outs=[gathered.opt()])
                        combT = ctp.tile([P, NCORES * B], _DT_BF, tag="combT",
                                         name="combT")
                        nc.sync.dma_start_transpose(combT[:, :],
                                                    gathered[:, :])
                        for c2 in range(NCORES):
                            kc = (k * 2 + br) * NCORES + c2
                            lhs = combT[:, c2 * B:(c2 + 1) * B]
                            goff = (br * NCORES + c2) * 4 * P
                            nc.tensor.matmul(
                                pg_t[:, :], lhsT=lhs,
                                rhs=gw_k_tile[:, goff:goff + 4 * P],
                                start=False, stop=(kc == 47))
                        # prefetch next k's masks after the R branch
                    # prefetch next k AFTER both branches' extract bounces
                    # so the slot-waiting bulk DMAs queue behind the
                    # latency-critical bounce DMAs on the SP ring, not ahead
                    if k + 1 < K:
                        m_tiles = load_mask_tiles(k + 1)
                        gw_k_tile = load_gw_k(k + 1, gwp)

                # ---- LSTM cell ----
                f_t = sml.tile([B, G], _DT_F32, tag="f", name="f")
                i_t = sml.tile([B, G], _DT_F32, tag="i", name="i")
                o_t = sml.tile([B, G], _DT_F32, tag="o", name="o")
                C_t = sml.tile([B, G], _DT_F32, tag="C", name="C")
                nc.scalar.activation(f_t[:, :], pg_t[:, 0:G], AF.Sigmoid)
                nc.scalar.activation(i_t[:, :], pg_t[:, G:2 * G], AF.Sigmoid)
                nc.scalar.activation(o_t[:, :], pg_t[:, 2 * G:3 * G],
                                     AF.Sigmoid)
                nc.scalar.activation(C_t[:, :], pg_t[:, 3 * G:4 * G], AF.Tanh)
                t1 = sml.tile([B, G], _DT_F32, tag="t1", name="t1")
                nc.vector.tensor_mul(t1[:, :], f_t[:, :], cs_t[:, :])
                t2 = sml.tile([B, G], _DT_F32, tag="t2", name="t2")
                nc.vector.tensor_mul(t2[:, :], i_t[:, :], C_t[:, :])
                cell = sml.tile([B, G], _DT_F32, tag="cell", name="cell")
                nc.vector.tensor_add(cell[:, :], t1[:, :], t2[:, :])
                tc_t = sml.tile([B, G], _DT_F32, tag="tc", name="tcl")
                nc.scalar.activation(tc_t[:, :], cell[:, :], AF.Tanh)
                hid = sml.tile([B, G], _DT_F32, tag="hid", name="hid")
                nc.vector.tensor_mul(hid[:, :], o_t[:, :], tc_t[:, :])
                nc.sync.dma_start(out=p_out[0, :, :], in_=hid[:, :])
                nc.sync.dma_start(out=p_out[1, :, :], in_=cell[:, :])

    nc.compile()
    return nc


def _bf(a):
    return np.ascontiguousarray(a.astype(BF16))


def prep_in_maps(input, R1_i, R2_i, R3_i, Hidden_State, Cell_State,
                 GC_R1_w, GC_R2_w, GC_R3_w, GC_NR1_w, GC_NR2_w, GC_NR3_w,
                 fl_w, fl_b, il_w, il_b, ol_w, ol_b, Cl_w, Cl_b):
    """Shard + relayout all inputs for the 8 cores (host side)."""
    input = np.asarray(input, np.float32)
    masks = [np.asarray(m, np.float32) for m in (R1_i, R2_i, R3_i)]
    hs = np.asarray(Hidden_State, np.float32)
    cs = np.asarray(Cell_State, np.float32)
    gcw = [np.asarray(w, np.float32) for w in
           (GC_R1_w, GC_R2_w, GC_R3_w, GC_NR1_w, GC_NR2_w, GC_NR3_w)]
    gates = [(np.asarray(w, np.float32), np.asarray(b, np.float32))
             for w, b in ((fl_w, fl_b), (il_w, il_b), (ol_w, ol_b),
                          (Cl_w, Cl_b))]

    # replicated tensors
    xT = _bf(input.T.reshape(FC, P, B).transpose(1, 0, 2).reshape(P, FC * B))
    hT = hs.T.reshape(FC, P, B).transpose(1, 0, 2).reshape(P, FC * B)
    bias_blk = np.zeros((P, B), np.float32)
    bias_blk[0, :] = 1.0
    hp = _bf(np.concatenate([hT, bias_blk], axis=1))

    # gathered-feature order for gate weight rows: per (mask k, branch br)
    # the AllGather emits rank-major rows (core, b) with columns g;
    # chunk kc = (k*2+br)*8 + core, feature = (br*3+k)*F + core*G + g
    feat = np.empty(48 * P, np.int64)
    idx = 0
    for k in range(K):
        for br in range(2):
            for c2 in range(NCORES):
                base = (br * 3 + k) * F + c2 * G
                feat[idx:idx + P] = np.arange(base, base + P)
                idx += P
    h_feat = np.arange(6 * F, 7 * F)

    in_maps = []
    for c in range(NCORES):
        gsl = slice(c * G, (c + 1) * G)
        m = np.empty((K, FC, P, B * G), BF16)
        for k in range(K):
            t = masks[k][:, :, gsl].transpose(1, 0, 2)      # [F, B, G]
            m[k] = _bf(t.reshape(FC, P, B * G))
        w6 = np.empty((K, P, 2 * FC * G), BF16)
        for w_i, W in enumerate(gcw):
            k, br = (w_i, 0) if w_i < 3 else (w_i - 3, 1)
            blk = W[:, gsl].reshape(FC, P, G).transpose(1, 0, 2)
            w6[k, :, br * FC * G:(br + 1) * FC * G] = _bf(
                blk.reshape(P, FC * G))
        allg = np.empty((4, NKC, P, P), np.float32)       # [gate, kc, kk, m]
        for g_i, (W, bv) in enumerate(gates):
            Wc = W[gsl, :]                                   # [G(out), 7F]
            gpart = Wc[:, feat].T.reshape(48, P, P)          # [kc, kk, m]
            hpart = Wc[:, h_feat].T.reshape(FC, P, P)
            bias_chunk = np.zeros((1, P, P), np.float32)
            bias_chunk[0, 0, :] = bv[gsl]
            allg[g_i] = np.concatenate([gpart, hpart, bias_chunk], axis=0)
        gw = _bf(allg.transpose(2, 1, 0, 3).reshape(P, NKC * 4 * P))
        in_maps.append({
            "m": m, "w6": w6, "xT": xT, "hp": hp, "gw": gw,
            "cs": np.ascontiguousarray(cs[:, gsl]),
        })
    return in_maps


_cached_nc = None


def _to_np(v):
    try:
        return np.asarray(v)
    except Exception:
        import jax
        return np.asarray(jax.device_put(v, jax.devices("cpu")[0]))


def kernel(**inputs):
    """Full inputs in, full outputs out. Shards across 8 NeuronCores."""
    global _cached_nc
    inputs = {k: _to_np(v) for k, v in inputs.items()}
    # NR1_i/NR2_i/NR3_i are accepted but unused (reference reuses R masks)
    args = {k: inputs[k] for k in (
        "input", "R1_i", "R2_i", "R3_i", "Hidden_State", "Cell_State",
        "GC_R1_w", "GC_R2_w", "GC_R3_w", "GC_NR1_w", "GC_NR2_w", "GC_NR3_w",
        "fl_w", "fl_b", "il_w", "il_b", "ol_w", "ol_b", "Cl_w", "Cl_b")}
    in_maps = prep_in_maps(**args)
    if _cached_nc is None:
        _cached_nc = build_nc(reps=1)
    res = run_bass_kernel_spmd(_cached_nc, in_maps,
                               core_ids=list(range(NCORES)))
    hidden = np.empty((B, F), np.float32)
    cell = np.empty((B, F), np.float32)
    for c in range(NCORES):
        o = res.results[c]["out"]
        hidden[:, c * G:(c + 1) * G] = o[0]
        cell[:, c * G:(c + 1) * G] = o[1]
    return hidden, cell

